# revision 23
# baseline (speedup 1.0000x reference)
"""EuclideanDeconf kernel for 8x TRN2 NeuronCores (v2).

Computes out[b, c] = (2/D) * x @ W.T - ||x||^2/D - ||W||^2/D
for x [16384, 1024] f32, W [2048, 1024] f32 -> out [16384, 2048] f32.

Sharding: data-parallel over batch. Each core gets 2048 rows of x and the
full W. Host work is layout/dtype-only (transpose / tile / cast / concat);
all FLOPs (matmul, row/col norms, combine) run on device.

v2 changes vs the 126us v1:
  - y is stored fp16 (16->8 MB DMA out) and epilogue pass-2 runs on DVE in
    fp16 (2x DVE rate). t (pass-1 out) and w2rep are fp16 too.
  - x arrives pre-cast: xP e4m3 [128,4,8,512] for the matmul (no on-device
    cast) and xR bf16 [128,16,1024] row-major for the x^2 path.
  - x2 column per b-tile comes from ONE fused ACT Square activation with
    accum_out (free-dim f32 accumulate) on a row-major bf16 x copy,
    replacing v1's gpsimd squares + DVE tree + PE dot + ACT copy. (DVE
    tensor_tensor_reduce computes the same thing in one op but crashes
    the device, so ACT it is.) The epilogue is sign-flipped so no negate
    is needed: u = -cross + x2 (pass-1), y = (-w2) - u (pass-2).
  - host pre-tiles all HBM layouts so every DMA is 128 runs of 4-16 KB
    (v1 moved 26 MB in ~7k small runs; v2 moves 14 MB in ~1.3k big runs).
  - W^2 squares split DVE (q0,q1,q3) / GPSIMD (q2); w2 reduce uses fp8
    DoubleRow (16 matmuls, M=16 for the dual-fp8 LDWEIGHTS ISA rule).
  - pass-1 halves split ACT (Identity+bias) / DVE (tensor_scalar) to
    balance the two engines.
  - emission order hand-scheduled (in-order queues) so the PE never waits
    on DVE/ACT backlogs: W DMA'd in cj-quarters pacing tile-0, w2 chains
    interleaved with tiles 1-5, pass-2 col-half h1 deferred until its
    w2rep half exists.

Numerics: cross term via e4m3 (W prescaled x16; x unscaled), x2 via bf16
squares in f32 accum, w2 via e4m3 squares, y in fp16. Host-measured:
norm rel err ~3e-4, max rel err ~1.5e-3 (gate 2e-2).
"""

import os as _os

import numpy as np
import ml_dtypes

B, D, C = 16384, 1024, 2048
NCORES = 8
BSH = B // NCORES          # 2048 rows of x per core
P = 128
NT = BSH // P              # 16 b-tiles per core
NQ = 4                     # cj / W quarters of 512 cols

_CACHE = {}

WARM = int(_os.environ.get("K_WARM", "3"))
PASS2 = _os.environ.get("K_PASS2", "tt")  # "tt" | "scan"


def _build_nc():
    import concourse.tile as tile
    import concourse.mybir as mybir
    import concourse.bass as bass
    from concourse import bacc

    f32 = mybir.dt.float32
    f16 = mybir.dt.float16
    bf16 = mybir.dt.bfloat16
    fp8 = mybir.dt.float8e4
    PSUM = bass.MemorySpace.PSUM
    Identity = mybir.ActivationFunctionType.Identity
    Copy = mybir.ActivationFunctionType.Copy
    Square = mybir.ActivationFunctionType.Square
    MULT = mybir.AluOpType.mult
    ADD = mybir.AluOpType.add
    SUB = mybir.AluOpType.subtract
    BYP = mybir.AluOpType.bypass
    DR = mybir.MatmulPerfMode.DoubleRow

    cross_scale = 2.0 / D / 16.0      # W host-prescaled by 16
    w2_scale = 1.0 / D / 256.0
    x2_sqrt_scale = 0.03125           # Square(x * 2^-5) = x^2 / 1024

    nc = bacc.Bacc(
        "TRN2",
        target_bir_lowering=False,
        debug=False,
        enable_asserts=False,
    )
    xP = nc.dram_tensor("xP", [P, NQ, 8, 512], fp8, kind="ExternalInput").ap()
    xR = nc.dram_tensor("xR", [P, NT, D], bf16, kind="ExternalInput").ap()
    wP = nc.dram_tensor("wP", [P, NQ, 8, 512], fp8, kind="ExternalInput").ap()
    yD = nc.dram_tensor("y", [P, NT, C], f16, kind="ExternalOutput").ap()

    # y store groups: tiles [0-3], [4-7], [8-11], [12-13], [14], [15]
    GROUPS = [(0, 4), (4, 4), (8, 4), (12, 2), (14, 1), (15, 1)]
    g_of = {}
    for g, (j0, nj) in enumerate(GROUPS):
        for j in range(j0, j0 + nj):
            g_of[j] = g

    with tile.TileContext(nc) as tc:
        with (
            tc.tile_pool(name="consts", bufs=1) as cpool,
            tc.tile_pool(name="wpool", bufs=1) as wpool,
            tc.tile_pool(name="xpool", bufs=1) as xpool,
            tc.tile_pool(name="scr", bufs=3) as spool,
            tc.tile_pool(name="x2pool", bufs=16) as x2pool,
            tc.tile_pool(name="epool", bufs=14) as epool,
            tc.tile_pool(name="ypool", bufs=3) as ypool,
            tc.tile_pool(name="pmain", bufs=4, space=PSUM) as pmain,
        ):
            # ---- tiles (one per DMA chunk so consumers wait only on
            # their own chunk's transfer, not the whole tensor) ----
            xbf = [xpool.tile([P, 8, 512], fp8, name=f"xbf{c}")
                   for c in range(4)]
            xrr = [xpool.tile([P, 4, D], bf16, name=f"xrr{g}")
                   for g in range(4)]
            wbf = [wpool.tile([P, 8, 512], fp8, name=f"wbf{q}")
                   for q in range(4)]
            wsq = [wpool.tile([P, 8, 512], fp8, name=f"wsq{q}")
                   for q in range(4)]
            w2row = wpool.tile([1, C], bf16)
            w2rep = wpool.tile([P, C], f16)

            # ---- input DMA on the sync ring; tile-0's two chunks are
            # k-halved and lead the queue so its first matmuls can start
            # as early as possible ----
            nc.sync.dma_start(wbf[0][:, 0:4], wP[:, 0, 0:4])
            nc.sync.dma_start(xbf[0][:, 0:4], xP[:, 0, 0:4])
            nc.sync.dma_start(wbf[0][:, 4:8], wP[:, 0, 4:8])
            nc.sync.dma_start(xbf[0][:, 4:8], xP[:, 0, 4:8])
            nc.sync.dma_start(wbf[1][:], wP[:, 1])
            nc.sync.dma_start(wbf[2][:], wP[:, 2])
            nc.sync.dma_start(wbf[3][:], wP[:, 3])
            nc.sync.dma_start(xrr[0][:], xR[:, 0:4])
            nc.sync.dma_start(xbf[1][:], xP[:, 1])
            nc.sync.dma_start(xrr[1][:], xR[:, 4:8])
            nc.sync.dma_start(xbf[2][:], xP[:, 2])
            nc.sync.dma_start(xrr[2][:], xR[:, 8:12])
            nc.sync.dma_start(xbf[3][:], xP[:, 3])
            nc.sync.dma_start(xrr[3][:], xR[:, 12:16])

            # ---- consts ----
            # [K-pair, M=16]: DoubleRow fp8 LDWEIGHTS requires M >= 16;
            # all 16 output rows hold the same sums, row 0 is read.
            negones_dr = cpool.tile([P, 2, 16], fp8)
            nc.gpsimd.memset(negones_dr[:], -1.0)
            ones1_b = cpool.tile([1, P], bf16)
            nc.gpsimd.memset(ones1_b[:], 1.0)
            warm_b = cpool.tile([P, 512], bf16)
            nc.gpsimd.memset(warm_b[:], 0.0)
            warm1 = cpool.tile([1, 1], f32)

            # ACT warm: function-table DMA off the critical path
            nc.scalar.activation(warm1[:], warm_b[0:1, 0:1], Identity,
                                 bias=0.0, scale=1.0)

            # PE warmup: release the HAM clock-gate early (borrows a pmain
            # psum buf; retired before tile-0 needs it)
            warm_ps = pmain.tile([P, 1024], f32, tag="ps", name="warmps")
            for _ in range(WARM):
                nc.tensor.matmul(warm_ps[:, 0:512], warm_b[:, 0:P], warm_b[:],
                                 start=True, stop=True)

            # ---- emission helpers ----
            psums = {}
            x2cs = {}
            ttiles = {}
            ybufs = {}

            def mms(j, q_outer=False):
                cq, jl = divmod(j, 4)
                ps0 = pmain.tile([P, 1024], f32, tag="ps", name=f"ps{j}a")
                ps1 = pmain.tile([P, 1024], f32, tag="ps", name=f"ps{j}b")
                psums[j] = (ps0, ps1)
                pss = (ps0, ps0, ps1, ps1)
                order = ([(q, k2) for q in range(4) for k2 in range(4)]
                         if q_outer else
                         [(q, k2) for k2 in range(4) for q in range(4)])
                for q, k2 in order:
                    nc.tensor.matmul(
                        pss[q][:, (q % 2) * 512:(q % 2) * 512 + 512],
                        xbf[cq][:, 2 * k2:2 * k2 + 2, jl * P:(jl + 1) * P],
                        wbf[q][:, 2 * k2:2 * k2 + 2, :],
                        start=(k2 == 0),
                        stop=(k2 == 3),
                        perf_mode=DR,
                    )

            def x2sq(j):
                """x2c_j = +sum(x_j^2)/D via ACT Square + free-dim accum."""
                scr = spool.tile([P, 1024], bf16, tag="scr", name=f"scr{j}")
                x2c = x2pool.tile([P, 1], f32, tag="x2c", name=f"x2c{j}")
                nc.scalar.activation(scr[:], xrr[j // 4][:, j % 4], Square,
                                     bias=0.0, scale=x2_sqrt_scale,
                                     accum_out=x2c[:])
                x2cs[j] = x2c

            ACT_P1 = {(j, 0) for j in (0, 1, 2, 4, 6, 8, 10, 12, 13, 14, 15)}
            ACT_P1 |= {(0, 1), (1, 1), (2, 1)}

            def p1(j):
                """u = -cross + x2 per half; split ACT/DVE by table."""
                ps0, ps1 = psums.pop(j)
                x2c = x2cs.pop(j)
                th = []
                for h, psh in enumerate((ps0, ps1)):
                    t = epool.tile([P, 1024], f16, tag="t", name=f"t{j}_{h}")
                    if (j, h) in ACT_P1:
                        nc.scalar.activation(t[:], psh[:], Identity,
                                             bias=x2c[:],
                                             scale=-cross_scale)
                    else:
                        nc.vector.tensor_scalar(t[:], psh[:], -cross_scale,
                                                x2c[:], op0=MULT, op1=ADD)
                    th.append(t)
                ttiles[j] = th

            def p2(j, h):
                """y = (-w2) - u on DVE (fp16)."""
                t = ttiles[j][h]
                g = g_of[j]
                if g not in ybufs:
                    ybufs[g] = ypool.tile([P, 4, C], f16, tag="yb",
                                          name=f"yb{g}")
                jo = j - GROUPS[g][0]
                ysl = ybufs[g][:, jo, h * 1024:(h + 1) * 1024]
                w2sl = w2rep[:, h * 1024:(h + 1) * 1024]
                if PASS2 == "scan":
                    nc.vector.tensor_tensor_scan(ysl, w2sl, t[:], 0.0,
                                                 op0=BYP, op1=SUB)
                else:
                    nc.vector.tensor_tensor(ysl, w2sl, t[:], op=SUB)

            def wsq_dve(q):
                nc.vector.tensor_tensor(wsq[q][:], wbf[q][:], wbf[q][:],
                                        op=MULT)

            def wsq_gps(q):
                nc.gpsimd.tensor_tensor(wsq[q][:], wbf[q][:], wbf[q][:],
                                        op=MULT)

            def wsq_act(q):
                nc.scalar.activation(wsq[q][:], wbf[q][:], Square)

            w2ps = {}

            def w2red(q):
                wp = pmain.tile([P, 1024], f32, tag="ps", name=f"w2ps{q}")
                for k2 in range(4):
                    nc.tensor.matmul(
                        wp[0:16, 0:512],
                        negones_dr[:],
                        wsq[q][:, 2 * k2:2 * k2 + 2, :],
                        start=(k2 == 0),
                        stop=(k2 == 3),
                        perf_mode=DR,
                    )
                w2ps[q] = wp

            def w2row_q(q):
                wp = w2ps.pop(q)
                nc.scalar.activation(w2row[:, q * 512:(q + 1) * 512],
                                     wp[0:1, 0:512], Copy, bias=0.0,
                                     scale=w2_scale)

            w2rp = {}

            def w2rp_q(q):
                wp = pmain.tile([P, 1024], f32, tag="ps", name=f"w2rp{q}")
                nc.tensor.matmul(wp[:, 0:512], ones1_b[:],
                                 w2row[:, q * 512:(q + 1) * 512],
                                 start=True, stop=True)
                w2rp[q] = wp

            def w2rep_q(q):
                wp = w2rp.pop(q)
                nc.scalar.activation(w2rep[:, q * 512:(q + 1) * 512],
                                     wp[:, 0:512], Copy, bias=0.0, scale=1.0)

            def store(g):
                j0, nj = GROUPS[g]
                yb = ybufs.pop(g)
                nc.sync.dma_start(yD[:, j0:j0 + nj, :], yb[:, 0:nj, :])

            def store_h(g, h):
                j0, nj = GROUPS[g]
                yb = ybufs[g] if h == 0 else ybufs.pop(g)
                nc.sync.dma_start(
                    yD[:, j0:j0 + nj, h * 1024:(h + 1) * 1024],
                    yb[:, 0:nj, h * 1024:(h + 1) * 1024],
                )

            # ---- scheduled emission ----
            wsq_act(0)
            wsq_gps(2); wsq_gps(3)
            mms(0, q_outer=True)
            x2sq(0)
            p1(0)
            mms(1)
            x2sq(1); x2sq(2); x2sq(3)
            p1(1)
            mms(2)
            p1(2)
            wsq_act(1)
            mms(3)
            x2sq(4)
            w2red(0); w2row_q(0)
            p1(3)
            mms(4)
            x2sq(5)
            w2red(1); w2row_q(1)
            w2rp_q(0); w2rep_q(0)
            p1(4)
            mms(5)
            x2sq(6)
            w2red(2); w2row_q(2)
            w2rp_q(1); w2rep_q(1)
            p1(5)
            p2(0, 0); p2(1, 0)
            mms(6)
            x2sq(7)
            w2red(3); w2row_q(3)
            w2rp_q(2); w2rep_q(2)
            p1(6)
            p2(2, 0); p2(3, 0); p2(4, 0)
            mms(7)
            x2sq(8)
            w2rp_q(3); w2rep_q(3)
            p1(7)
            p2(5, 0); p2(6, 0); p2(0, 1); p2(1, 1)
            mms(8)
            x2sq(9)
            p1(8)
            p2(7, 0); p2(2, 1); p2(3, 1)
            store(0)
            mms(9)
            x2sq(10)
            p1(9)
            p2(8, 0); p2(8, 1); p2(4, 1); p2(5, 1)
            mms(10)
            x2sq(11)
            p1(10)
            p2(9, 0); p2(9, 1); p2(6, 1); p2(7, 1)
            store(1)
            mms(11)
            x2sq(12)
            p1(11)
            p2(10, 0); p2(10, 1)
            mms(12)
            x2sq(13)
            p1(12)
            p2(11, 0); p2(11, 1)
            mms(13)
            x2sq(14)
            p1(13)
            p2(12, 0); p2(12, 1)
            store(2)
            mms(14)
            x2sq(15)
            p1(14)
            p2(13, 0); p2(13, 1)
            store(3)
            mms(15)
            p2(14, 0)
            store_h(4, 0)
            p2(14, 1)
            store_h(4, 1)
            # tile 15 epilogue at quarter granularity: ACT p1 quarters feed
            # DVE p2 quarters concurrently to shorten the drain tail
            ps0_15, ps1_15 = psums.pop(15)
            x2c_15 = x2cs.pop(15)
            yb5 = ypool.tile([P, 4, C], f16, tag="yb", name="yb5")
            for qq in range(4):
                psh = (ps0_15, ps1_15)[qq // 2]
                sl = slice((qq % 2) * 512, (qq % 2) * 512 + 512)
                csl = slice(qq * 512, (qq + 1) * 512)
                tq = epool.tile([P, 512], f16, tag="tq", bufs=4,
                                name=f"t15_{qq}")
                nc.scalar.activation(tq[:], psh[:, sl], Identity,
                                     bias=x2c_15[:], scale=-cross_scale)
                nc.vector.tensor_tensor(yb5[:, 0, csl], w2rep[:, csl],
                                        tq[:], op=SUB)
            nc.sync.dma_start(yD[:, 15:16, 0:1024], yb5[:, 0:1, 0:1024])
            nc.sync.dma_start(yD[:, 15:16, 1024:2048], yb5[:, 0:1, 1024:2048])

    nc.compile()
    return nc


def _get_nc():
    if "nc" not in _CACHE:
        _CACHE["nc"] = _build_nc()
    return _CACHE["nc"]


def _prep_inputs(x, W):
    x = np.ascontiguousarray(x, dtype=np.float32)
    W = np.ascontiguousarray(W, dtype=np.float32)
    e4m3 = ml_dtypes.float8_e4m3
    bf16 = ml_dtypes.bfloat16
    # wP[p, q, k, c'] = 16*W[q*512+c', k*128+p]
    w8 = (W * np.float32(16.0)).astype(e4m3)
    wPm = np.ascontiguousarray(
        w8.reshape(4, 512, 8, P).transpose(3, 0, 2, 1)
    )
    in_maps = []
    for i in range(NCORES):
        xs = x[i * BSH:(i + 1) * BSH]
        x8 = xs.astype(e4m3)
        # xP[p, c, k, b'] = x8[c*512+b', k*128+p]
        xPm = np.ascontiguousarray(
            x8.T.reshape(8, P, 4, 512).transpose(1, 2, 0, 3)
        )
        xb = xs.astype(bf16)
        # xR[p, j, d] = xb[j*128+p, d]
        xRm = np.ascontiguousarray(
            xb.reshape(NT, P, D).transpose(1, 0, 2)
        )
        in_maps.append({"xP": xPm, "xR": xRm, "wP": wPm})
    return in_maps


def run(x, W, trace=False, **trace_kwargs):
    """Run on the 8 cores; returns (out [B, C] f32, BassKernelResults)."""
    from concourse import bass_utils

    nc = _get_nc()
    in_maps = _prep_inputs(x, W)
    res = bass_utils.run_bass_kernel_spmd(
        nc, in_maps, core_ids=list(range(NCORES)), trace=trace, **trace_kwargs
    )
    outs = []
    for r in res.results:
        yt = r["y"]  # [128, 16, 2048] fp16
        outs.append(
            np.ascontiguousarray(yt.transpose(1, 0, 2))
            .reshape(BSH, C)
            .astype(np.float32)
        )
    out = np.concatenate(outs, axis=0)
    return out, res


def kernel(x, W, task_id=None, **_unused):
    out, _ = run(np.asarray(x), np.asarray(W), trace=False)
    return out


# revision 24
# speedup vs baseline: 1.0212x; 1.0212x over previous
"""EuclideanDeconf kernel for 8x TRN2 NeuronCores (v2).

Computes out[b, c] = (2/D) * x @ W.T - ||x||^2/D - ||W||^2/D
for x [16384, 1024] f32, W [2048, 1024] f32 -> out [16384, 2048] f32.

Sharding: data-parallel over batch. Each core gets 2048 rows of x and the
full W. Host work is layout/dtype-only (transpose / tile / cast / concat);
all FLOPs (matmul, row/col norms, combine) run on device.

v2 changes vs the 126us v1:
  - y is stored fp16 (16->8 MB DMA out) and epilogue pass-2 runs on DVE in
    fp16 (2x DVE rate). t (pass-1 out) and w2rep are fp16 too.
  - x arrives pre-cast: xP e4m3 [128,4,8,512] for the matmul (no on-device
    cast) and xR bf16 [128,16,1024] row-major for the x^2 path.
  - x2 column per b-tile comes from ONE fused ACT Square activation with
    accum_out (free-dim f32 accumulate) on a row-major bf16 x copy,
    replacing v1's gpsimd squares + DVE tree + PE dot + ACT copy. (DVE
    tensor_tensor_reduce computes the same thing in one op but crashes
    the device, so ACT it is.) The epilogue is sign-flipped so no negate
    is needed: u = -cross + x2 (pass-1), y = (-w2) - u (pass-2).
  - host pre-tiles all HBM layouts so every DMA is 128 runs of 4-16 KB
    (v1 moved 26 MB in ~7k small runs; v2 moves 14 MB in ~1.3k big runs).
  - W^2 squares split DVE (q0,q1,q3) / GPSIMD (q2); w2 reduce uses fp8
    DoubleRow (16 matmuls, M=16 for the dual-fp8 LDWEIGHTS ISA rule).
  - pass-1 halves split ACT (Identity+bias) / DVE (tensor_scalar) to
    balance the two engines.
  - emission order hand-scheduled (in-order queues) so the PE never waits
    on DVE/ACT backlogs: W DMA'd in cj-quarters pacing tile-0, w2 chains
    interleaved with tiles 1-5, pass-2 col-half h1 deferred until its
    w2rep half exists.

Numerics: cross term via e4m3 (W prescaled x16; x unscaled), x2 via bf16
squares in f32 accum, w2 via e4m3 squares, y in fp16. Host-measured:
norm rel err ~3e-4, max rel err ~1.5e-3 (gate 2e-2).
"""

import os as _os

import numpy as np
import ml_dtypes

B, D, C = 16384, 1024, 2048
NCORES = 8
BSH = B // NCORES          # 2048 rows of x per core
P = 128
NT = BSH // P              # 16 b-tiles per core
NQ = 4                     # cj / W quarters of 512 cols

_CACHE = {}

WARM = int(_os.environ.get("K_WARM", "8"))
PASS2 = _os.environ.get("K_PASS2", "tt")  # "tt" | "scan"


def _build_nc():
    import concourse.tile as tile
    import concourse.mybir as mybir
    import concourse.bass as bass
    from concourse import bacc

    f32 = mybir.dt.float32
    f16 = mybir.dt.float16
    bf16 = mybir.dt.bfloat16
    fp8 = mybir.dt.float8e4
    PSUM = bass.MemorySpace.PSUM
    Identity = mybir.ActivationFunctionType.Identity
    Copy = mybir.ActivationFunctionType.Copy
    Square = mybir.ActivationFunctionType.Square
    MULT = mybir.AluOpType.mult
    ADD = mybir.AluOpType.add
    SUB = mybir.AluOpType.subtract
    BYP = mybir.AluOpType.bypass
    DR = mybir.MatmulPerfMode.DoubleRow

    cross_scale = 2.0 / D / 16.0      # W host-prescaled by 16
    w2_scale = 1.0 / D / 256.0
    x2_sqrt_scale = 0.03125           # Square(x * 2^-5) = x^2 / 1024

    nc = bacc.Bacc(
        "TRN2",
        target_bir_lowering=False,
        debug=False,
        enable_asserts=False,
    )
    xP = nc.dram_tensor("xP", [P, NQ, 8, 512], fp8, kind="ExternalInput").ap()
    xR = nc.dram_tensor("xR", [P, NT, D], bf16, kind="ExternalInput").ap()
    wP = nc.dram_tensor("wP", [P, NQ, 8, 512], fp8, kind="ExternalInput").ap()
    yD = nc.dram_tensor("y", [P, NT, C], f16, kind="ExternalOutput").ap()

    # y store groups, finer near the end so the store tail overlaps compute
    GROUPS = [(0, 4), (4, 4), (8, 2), (10, 2), (12, 2), (14, 1), (15, 1)]
    g_of = {}
    for g, (j0, nj) in enumerate(GROUPS):
        for j in range(j0, j0 + nj):
            g_of[j] = g

    with tile.TileContext(nc) as tc:
        with (
            tc.tile_pool(name="consts", bufs=1) as cpool,
            tc.tile_pool(name="wpool", bufs=1) as wpool,
            tc.tile_pool(name="xpool", bufs=1) as xpool,
            tc.tile_pool(name="scr", bufs=3) as spool,
            tc.tile_pool(name="x2pool", bufs=16) as x2pool,
            tc.tile_pool(name="epool", bufs=14) as epool,
            tc.tile_pool(name="ypool", bufs=3) as ypool,
            tc.tile_pool(name="pmain", bufs=4, space=PSUM) as pmain,
        ):
            # ---- tiles (one per DMA chunk so consumers wait only on
            # their own chunk's transfer, not the whole tensor) ----
            xbf = [xpool.tile([P, 8, 512], fp8, name=f"xbf{c}")
                   for c in range(4)]
            xrr = [xpool.tile([P, 4, D], bf16, name=f"xrr{g}")
                   for g in range(4)]
            wbf = [wpool.tile([P, 8, 512], fp8, name=f"wbf{q}")
                   for q in range(4)]
            wsq = [wpool.tile([P, 8, 512], fp8, name=f"wsq{q}")
                   for q in range(4)]
            w2row = wpool.tile([1, C], bf16)
            w2rep = wpool.tile([P, C], f16)

            # ---- input DMA on the sync ring; tile-0's two chunks are
            # k-halved and lead the queue so its first matmuls can start
            # as early as possible ----
            nc.sync.dma_start(wbf[0][:], wP[:, 0])
            nc.sync.dma_start(xbf[0][:], xP[:, 0])
            nc.sync.dma_start(wbf[1][:], wP[:, 1])
            nc.sync.dma_start(wbf[2][:], wP[:, 2])
            nc.sync.dma_start(wbf[3][:], wP[:, 3])
            nc.sync.dma_start(xrr[0][:], xR[:, 0:4])
            nc.sync.dma_start(xbf[1][:], xP[:, 1])
            nc.sync.dma_start(xrr[1][:], xR[:, 4:8])
            nc.sync.dma_start(xbf[2][:], xP[:, 2])
            nc.sync.dma_start(xrr[2][:], xR[:, 8:12])
            nc.sync.dma_start(xbf[3][:], xP[:, 3])
            nc.sync.dma_start(xrr[3][:], xR[:, 12:16])

            # ---- consts ----
            # [K-pair, M=16]: DoubleRow fp8 LDWEIGHTS requires M >= 16;
            # all 16 output rows hold the same sums, row 0 is read.
            negones_dr = cpool.tile([P, 2, 16], fp8)
            nc.gpsimd.memset(negones_dr[:], -1.0)
            ones1_b = cpool.tile([1, P], bf16)
            nc.gpsimd.memset(ones1_b[:], 1.0)
            warm_b = cpool.tile([P, 512], bf16)
            nc.gpsimd.memset(warm_b[:], 0.0)
            warm1 = cpool.tile([1, 1], f32)

            # ACT warm: function-table DMA off the critical path
            nc.scalar.activation(warm1[:], warm_b[0:1, 0:1], Identity,
                                 bias=0.0, scale=1.0)

            # PE warmup: release the HAM clock-gate early (borrows a pmain
            # psum buf; retired before tile-0 needs it)
            warm_ps = pmain.tile([P, 1024], f32, tag="ps", name="warmps")
            for _ in range(WARM):
                nc.tensor.matmul(warm_ps[:, 0:512], warm_b[:, 0:P], warm_b[:],
                                 start=True, stop=True)

            # ---- emission helpers ----
            psums = {}
            x2cs = {}
            ttiles = {}
            ybufs = {}

            def mms(j, q_outer=False):
                cq, jl = divmod(j, 4)
                ps0 = pmain.tile([P, 1024], f32, tag="ps", name=f"ps{j}a")
                ps1 = pmain.tile([P, 1024], f32, tag="ps", name=f"ps{j}b")
                psums[j] = (ps0, ps1)
                pss = (ps0, ps0, ps1, ps1)
                order = ([(q, k2) for q in range(4) for k2 in range(4)]
                         if q_outer else
                         [(q, k2) for k2 in range(4) for q in range(4)])
                for q, k2 in order:
                    nc.tensor.matmul(
                        pss[q][:, (q % 2) * 512:(q % 2) * 512 + 512],
                        xbf[cq][:, 2 * k2:2 * k2 + 2, jl * P:(jl + 1) * P],
                        wbf[q][:, 2 * k2:2 * k2 + 2, :],
                        start=(k2 == 0),
                        stop=(k2 == 3),
                        perf_mode=DR,
                    )

            def x2sq(j):
                """x2c_j = +sum(x_j^2)/D via ACT Square + free-dim accum."""
                scr = spool.tile([P, 1024], bf16, tag="scr", name=f"scr{j}")
                x2c = x2pool.tile([P, 1], f32, tag="x2c", name=f"x2c{j}")
                nc.scalar.activation(scr[:], xrr[j // 4][:, j % 4], Square,
                                     bias=0.0, scale=x2_sqrt_scale,
                                     accum_out=x2c[:])
                x2cs[j] = x2c

            ACT_P1 = {(j, 0) for j in (0, 1, 2, 4, 6, 8, 10, 12, 13, 14, 15)}
            ACT_P1 |= {(0, 1), (1, 1), (2, 1)}

            def p1(j):
                """u = -cross + x2 per half; split ACT/DVE by table."""
                ps0, ps1 = psums.pop(j)
                x2c = x2cs.pop(j)
                th = []
                for h, psh in enumerate((ps0, ps1)):
                    t = epool.tile([P, 1024], f16, tag="t", name=f"t{j}_{h}")
                    if (j, h) in ACT_P1:
                        nc.scalar.activation(t[:], psh[:], Identity,
                                             bias=x2c[:],
                                             scale=-cross_scale)
                    else:
                        nc.vector.tensor_scalar(t[:], psh[:], -cross_scale,
                                                x2c[:], op0=MULT, op1=ADD)
                    th.append(t)
                ttiles[j] = th

            def p2(j, h):
                """y = (-w2) - u on DVE (fp16)."""
                t = ttiles[j][h]
                g = g_of[j]
                if g not in ybufs:
                    ybufs[g] = ypool.tile([P, 4, C], f16, tag="yb",
                                          name=f"yb{g}")
                jo = j - GROUPS[g][0]
                ysl = ybufs[g][:, jo, h * 1024:(h + 1) * 1024]
                w2sl = w2rep[:, h * 1024:(h + 1) * 1024]
                if PASS2 == "scan":
                    nc.vector.tensor_tensor_scan(ysl, w2sl, t[:], 0.0,
                                                 op0=BYP, op1=SUB)
                else:
                    nc.vector.tensor_tensor(ysl, w2sl, t[:], op=SUB)

            def wsq_dve(q):
                nc.vector.tensor_tensor(wsq[q][:], wbf[q][:], wbf[q][:],
                                        op=MULT)

            def wsq_gps(q):
                nc.gpsimd.tensor_tensor(wsq[q][:], wbf[q][:], wbf[q][:],
                                        op=MULT)

            def wsq_act(q):
                nc.scalar.activation(wsq[q][:], wbf[q][:], Square)

            w2ps = {}

            def w2red(q):
                wp = pmain.tile([P, 1024], f32, tag="ps", name=f"w2ps{q}")
                for k2 in range(4):
                    nc.tensor.matmul(
                        wp[0:16, 0:512],
                        negones_dr[:],
                        wsq[q][:, 2 * k2:2 * k2 + 2, :],
                        start=(k2 == 0),
                        stop=(k2 == 3),
                        perf_mode=DR,
                    )
                w2ps[q] = wp

            def w2row_q(q):
                wp = w2ps.pop(q)
                nc.scalar.activation(w2row[:, q * 512:(q + 1) * 512],
                                     wp[0:1, 0:512], Copy, bias=0.0,
                                     scale=w2_scale)

            w2rp = {}

            def w2rp_q(q):
                wp = pmain.tile([P, 1024], f32, tag="ps", name=f"w2rp{q}")
                nc.tensor.matmul(wp[:, 0:512], ones1_b[:],
                                 w2row[:, q * 512:(q + 1) * 512],
                                 start=True, stop=True)
                w2rp[q] = wp

            def w2rep_q(q):
                wp = w2rp.pop(q)
                nc.scalar.activation(w2rep[:, q * 512:(q + 1) * 512],
                                     wp[:, 0:512], Copy, bias=0.0, scale=1.0)

            def store(g):
                j0, nj = GROUPS[g]
                yb = ybufs.pop(g)
                nc.sync.dma_start(yD[:, j0:j0 + nj, :], yb[:, 0:nj, :])

            def store_h(g, h):
                j0, nj = GROUPS[g]
                yb = ybufs[g] if h == 0 else ybufs.pop(g)
                nc.sync.dma_start(
                    yD[:, j0:j0 + nj, h * 1024:(h + 1) * 1024],
                    yb[:, 0:nj, h * 1024:(h + 1) * 1024],
                )

            # ---- scheduled emission ----
            wsq_act(0)
            wsq_gps(2); wsq_gps(3)
            mms(0, q_outer=True)
            x2sq(0)
            p1(0)
            mms(1)
            x2sq(1); x2sq(2); x2sq(3)
            p1(1)
            mms(2)
            p1(2)
            wsq_act(1)
            mms(3)
            x2sq(4)
            w2red(0); w2row_q(0)
            p1(3)
            mms(4)
            x2sq(5)
            w2red(1); w2row_q(1)
            w2rp_q(0); w2rep_q(0)
            p1(4)
            mms(5)
            x2sq(6)
            w2red(2); w2row_q(2)
            w2rp_q(1); w2rep_q(1)
            p1(5)
            p2(0, 0); p2(1, 0)
            mms(6)
            x2sq(7)
            w2red(3); w2row_q(3)
            w2rp_q(2); w2rep_q(2)
            p1(6)
            p2(2, 0); p2(3, 0); p2(4, 0)
            mms(7)
            x2sq(8)
            w2rp_q(3); w2rep_q(3)
            p1(7)
            p2(5, 0); p2(6, 0); p2(0, 1); p2(1, 1)
            mms(8)
            x2sq(9)
            p1(8)
            p2(7, 0); p2(2, 1); p2(3, 1)
            store(0)
            mms(9)
            x2sq(10)
            p1(9)
            p2(8, 0); p2(8, 1); p2(4, 1); p2(5, 1)
            mms(10)
            x2sq(11)
            p1(10)
            p2(9, 0); p2(9, 1); p2(6, 1); p2(7, 1)
            store(1)
            mms(11)
            x2sq(12)
            p1(11)
            p2(10, 0); p2(10, 1)
            store(2)
            mms(12)
            x2sq(13)
            p1(12)
            p2(11, 0); p2(11, 1)
            store(3)
            mms(13)
            x2sq(14)
            p1(13)
            p2(12, 0); p2(12, 1)
            mms(14)
            x2sq(15)
            p1(14)
            p2(13, 0); p2(13, 1)
            store(4)
            mms(15)
            p2(14, 0)
            store_h(5, 0)
            p2(14, 1)
            store_h(5, 1)
            # tile 15 epilogue at quarter granularity: ACT p1 quarters feed
            # DVE p2 quarters concurrently to shorten the drain tail
            ps0_15, ps1_15 = psums.pop(15)
            x2c_15 = x2cs.pop(15)
            yb5 = ypool.tile([P, 4, C], f16, tag="yb", name="yb5")
            for qq in range(4):
                psh = (ps0_15, ps1_15)[qq // 2]
                sl = slice((qq % 2) * 512, (qq % 2) * 512 + 512)
                csl = slice(qq * 512, (qq + 1) * 512)
                tq = epool.tile([P, 512], f16, tag="tq", bufs=4,
                                name=f"t15_{qq}")
                nc.scalar.activation(tq[:], psh[:, sl], Identity,
                                     bias=x2c_15[:], scale=-cross_scale)
                nc.vector.tensor_tensor(yb5[:, 0, csl], w2rep[:, csl],
                                        tq[:], op=SUB)
            nc.sync.dma_start(yD[:, 15:16, 0:1024], yb5[:, 0:1, 0:1024])
            nc.sync.dma_start(yD[:, 15:16, 1024:2048], yb5[:, 0:1, 1024:2048])

    nc.compile()
    return nc


def _get_nc():
    if "nc" not in _CACHE:
        _CACHE["nc"] = _build_nc()
    return _CACHE["nc"]


def _prep_inputs(x, W):
    x = np.ascontiguousarray(x, dtype=np.float32)
    W = np.ascontiguousarray(W, dtype=np.float32)
    e4m3 = ml_dtypes.float8_e4m3
    bf16 = ml_dtypes.bfloat16
    # wP[p, q, k, c'] = 16*W[q*512+c', k*128+p]
    w8 = (W * np.float32(16.0)).astype(e4m3)
    wPm = np.ascontiguousarray(
        w8.reshape(4, 512, 8, P).transpose(3, 0, 2, 1)
    )
    in_maps = []
    for i in range(NCORES):
        xs = x[i * BSH:(i + 1) * BSH]
        x8 = xs.astype(e4m3)
        # xP[p, c, k, b'] = x8[c*512+b', k*128+p]
        xPm = np.ascontiguousarray(
            x8.T.reshape(8, P, 4, 512).transpose(1, 2, 0, 3)
        )
        xb = xs.astype(bf16)
        # xR[p, j, d] = xb[j*128+p, d]
        xRm = np.ascontiguousarray(
            xb.reshape(NT, P, D).transpose(1, 0, 2)
        )
        in_maps.append({"xP": xPm, "xR": xRm, "wP": wPm})
    return in_maps


def run(x, W, trace=False, **trace_kwargs):
    """Run on the 8 cores; returns (out [B, C] f32, BassKernelResults)."""
    from concourse import bass_utils

    nc = _get_nc()
    in_maps = _prep_inputs(x, W)
    res = bass_utils.run_bass_kernel_spmd(
        nc, in_maps, core_ids=list(range(NCORES)), trace=trace, **trace_kwargs
    )
    outs = []
    for r in res.results:
        yt = r["y"]  # [128, 16, 2048] fp16
        outs.append(
            np.ascontiguousarray(yt.transpose(1, 0, 2))
            .reshape(BSH, C)
            .astype(np.float32)
        )
    out = np.concatenate(outs, axis=0)
    return out, res


def kernel(x, W, task_id=None, **_unused):
    out, _ = run(np.asarray(x), np.asarray(W), trace=False)
    return out


# revision 25
# speedup vs baseline: 1.0397x; 1.0181x over previous
"""EuclideanDeconf kernel for 8x TRN2 NeuronCores (v2, 89.3us vs 126.3us v1).

Computes out[b, c] = (2/D) * x @ W.T - ||x||^2/D - ||W||^2/D
for x [16384, 1024] f32, W [2048, 1024] f32 -> out [16384, 2048] f32.

Sharding: data-parallel over batch. Each core gets 2048 rows of x and the
full W. Host work is layout/dtype-only (transpose / tile / cast / concat);
all FLOPs (matmul, row/col norms, combine) run on device.

Design (per core: 256 fp8-DoubleRow matmuls = 55us at 2.4GHz is the PE
floor; everything else is scheduled around keeping the PE dense):
  - x arrives pre-cast e4m3 in a PE-friendly tiled layout [128,4,8,512]
    (no on-device cast), plus a bf16 row-major copy [128,16,1024] for the
    x^2 path. W arrives e4m3 pre-scaled by 16 (avoids e4m3 subnormals),
    same tiling. All host layouts give DMA runs of 128 x 4-16 KB (v1
    moved 26 MB in ~7k small runs; v2 moves 14 MB in ~1.3k big runs).
  - y is stored fp16 (8 MB out instead of 16) in a tiled layout the host
    un-tiles; epilogue tensors (u, w2rep) are fp16 for 2x DVE rate.
  - x2 column per b-tile: ONE ACT Square activation with accum_out
    (free-dim f32 accumulate) on the row-major x copy, replacing v1's
    gpsimd squares + DVE tree + PE dot + ACT copy. (DVE
    tensor_tensor_reduce would do it too but crashes the device;
    tensor_scalar with accum_out fails to compile; scan is 4x slower
    than plain tensor_tensor on HW - all probed.)
  - epilogue is sign-flipped so nothing needs negating: pass-1
    u = -cross + x2 (scale+bias, split ACT/DVE by a static table to
    balance the engines), pass-2 y = (-w2) - u (DVE fp16 subtract at
    2x; w2rep is built negative by reducing wsq against -1 weights).
  - W^2 squares split ACT (q0,q1) / GPSIMD (q2) / DVE via fp8; w2 reduce
    on the PE as 16 fp8-DoubleRow matmuls (M=16: the dual-fp8 LDWEIGHTS
    ISA check rejects narrower stationary tiles), then a K=1 broadcast
    matmul replicates -w2 across partitions.
  - all queues are in-order, so emission order is the schedule: W in
    cj-quarters paces tile-0, w2 chains interleave with tiles 1-7
    (replicate one tile after its reduce so the PE never waits on ACT),
    pass-2 column-half h1 is deferred until its w2rep half exists, y
    stores shrink toward the end (4,4,2,2,2,1,1 tiles) and the last two
    stores go out as halves; tile 15's epilogue runs at quarter
    granularity with ACT feeding DVE to cut the drain tail.
  - 8 warmup matmuls fill the DMA-bound head; psum pool is 4 x
    [128,1024] with the w2 psums borrowing from it (all 8 banks).

Numerics: cross term via e4m3 (max rel ~6e-4), x2 via bf16 squares in
f32 accum, w2 via e4m3 squares, y in fp16. Measured on HW vs the f32
reference: norm rel err 3.2e-4, max rel err 1.5e-3 (gate 2e-2).

Measured progression (HW exec, 8 cores): v1 126.3us -> fp16+scan 140us
(scan pass-2 is slow) -> tt pass-2 95.4us -> rebalanced engines /
per-chunk DMA tiles / borrowed psum 90-91us -> final schedule 89.3us.
PE busy ~64-66us of the ~89us span; the rest is NEFF boot (~8us,
fixed), input-DMA ramp to the first matmuls (~4us), a clock-ramp window
whose initial state varies run to run (matmuls take 216ns warm vs 426ns
cold), and the drain tail (~6us).
"""

import os as _os

import numpy as np
import ml_dtypes

B, D, C = 16384, 1024, 2048
NCORES = 8
BSH = B // NCORES          # 2048 rows of x per core
P = 128
NT = BSH // P              # 16 b-tiles per core
NQ = 4                     # cj / W quarters of 512 cols

_CACHE = {}

WARM = int(_os.environ.get("K_WARM", "8"))
PASS2 = _os.environ.get("K_PASS2", "tt")  # "tt" | "scan"


def _build_nc():
    import concourse.tile as tile
    import concourse.mybir as mybir
    import concourse.bass as bass
    from concourse import bacc

    f32 = mybir.dt.float32
    f16 = mybir.dt.float16
    bf16 = mybir.dt.bfloat16
    fp8 = mybir.dt.float8e4
    PSUM = bass.MemorySpace.PSUM
    Identity = mybir.ActivationFunctionType.Identity
    Copy = mybir.ActivationFunctionType.Copy
    Square = mybir.ActivationFunctionType.Square
    MULT = mybir.AluOpType.mult
    ADD = mybir.AluOpType.add
    SUB = mybir.AluOpType.subtract
    BYP = mybir.AluOpType.bypass
    DR = mybir.MatmulPerfMode.DoubleRow

    cross_scale = 2.0 / D / 16.0      # W host-prescaled by 16
    w2_scale = 1.0 / D / 256.0
    x2_sqrt_scale = 0.03125           # Square(x * 2^-5) = x^2 / 1024

    nc = bacc.Bacc(
        "TRN2",
        target_bir_lowering=False,
        debug=False,
        enable_asserts=False,
    )
    xP = nc.dram_tensor("xP", [P, NQ, 8, 512], fp8, kind="ExternalInput").ap()
    xR = nc.dram_tensor("xR", [P, NT, D], bf16, kind="ExternalInput").ap()
    wP = nc.dram_tensor("wP", [P, NQ, 8, 512], fp8, kind="ExternalInput").ap()
    yD = nc.dram_tensor("y", [P, NT, C], f16, kind="ExternalOutput").ap()

    # y store groups, finer near the end so the store tail overlaps compute
    GROUPS = [(0, 4), (4, 4), (8, 2), (10, 2), (12, 2), (14, 1), (15, 1)]
    g_of = {}
    for g, (j0, nj) in enumerate(GROUPS):
        for j in range(j0, j0 + nj):
            g_of[j] = g

    with tile.TileContext(nc) as tc:
        with (
            tc.tile_pool(name="consts", bufs=1) as cpool,
            tc.tile_pool(name="wpool", bufs=1) as wpool,
            tc.tile_pool(name="xpool", bufs=1) as xpool,
            tc.tile_pool(name="scr", bufs=3) as spool,
            tc.tile_pool(name="x2pool", bufs=16) as x2pool,
            tc.tile_pool(name="epool", bufs=14) as epool,
            tc.tile_pool(name="ypool", bufs=3) as ypool,
            tc.tile_pool(name="pmain", bufs=4, space=PSUM) as pmain,
        ):
            # ---- tiles (one per DMA chunk so consumers wait only on
            # their own chunk's transfer, not the whole tensor) ----
            xbf = [xpool.tile([P, 8, 512], fp8, name=f"xbf{c}")
                   for c in range(4)]
            xrr = [xpool.tile([P, 4, D], bf16, name=f"xrr{g}")
                   for g in range(4)]
            wbf = [wpool.tile([P, 8, 512], fp8, name=f"wbf{q}")
                   for q in range(4)]
            wsq = [wpool.tile([P, 8, 512], fp8, name=f"wsq{q}")
                   for q in range(4)]
            w2row = wpool.tile([1, C], bf16)
            w2rep = wpool.tile([P, C], f16)

            # ---- input DMA on the sync ring; tile-0's two chunks are
            # k-halved and lead the queue so its first matmuls can start
            # as early as possible ----
            nc.sync.dma_start(wbf[0][:], wP[:, 0])
            nc.sync.dma_start(xbf[0][:], xP[:, 0])
            nc.sync.dma_start(wbf[1][:], wP[:, 1])
            nc.sync.dma_start(wbf[2][:], wP[:, 2])
            nc.sync.dma_start(wbf[3][:], wP[:, 3])
            nc.sync.dma_start(xrr[0][:], xR[:, 0:4])
            nc.sync.dma_start(xbf[1][:], xP[:, 1])
            nc.sync.dma_start(xrr[1][:], xR[:, 4:8])
            nc.sync.dma_start(xbf[2][:], xP[:, 2])
            nc.sync.dma_start(xrr[2][:], xR[:, 8:12])
            nc.sync.dma_start(xbf[3][:], xP[:, 3])
            nc.sync.dma_start(xrr[3][:], xR[:, 12:16])

            # ---- consts ----
            # [K-pair, M=16]: DoubleRow fp8 LDWEIGHTS requires M >= 16;
            # all 16 output rows hold the same sums, row 0 is read.
            negones_dr = cpool.tile([P, 2, 16], fp8)
            nc.gpsimd.memset(negones_dr[:], -1.0)
            ones1_b = cpool.tile([1, P], bf16)
            nc.gpsimd.memset(ones1_b[:], 1.0)
            warm_b = cpool.tile([P, 512], bf16)
            nc.gpsimd.memset(warm_b[:], 0.0)
            warm1 = cpool.tile([1, 1], f32)

            # ACT warm: function-table DMA off the critical path
            nc.scalar.activation(warm1[:], warm_b[0:1, 0:1], Identity,
                                 bias=0.0, scale=1.0)

            # PE warmup: release the HAM clock-gate early (borrows a pmain
            # psum buf; retired before tile-0 needs it)
            warm_ps = pmain.tile([P, 1024], f32, tag="ps", name="warmps")
            for _ in range(WARM):
                nc.tensor.matmul(warm_ps[:, 0:512], warm_b[:, 0:P], warm_b[:],
                                 start=True, stop=True)

            # ---- emission helpers ----
            psums = {}
            x2cs = {}
            ttiles = {}
            ybufs = {}

            def mms(j, q_outer=False):
                cq, jl = divmod(j, 4)
                ps0 = pmain.tile([P, 1024], f32, tag="ps", name=f"ps{j}a")
                ps1 = pmain.tile([P, 1024], f32, tag="ps", name=f"ps{j}b")
                psums[j] = (ps0, ps1)
                pss = (ps0, ps0, ps1, ps1)
                order = ([(q, k2) for q in range(4) for k2 in range(4)]
                         if q_outer else
                         [(q, k2) for k2 in range(4) for q in range(4)])
                for q, k2 in order:
                    nc.tensor.matmul(
                        pss[q][:, (q % 2) * 512:(q % 2) * 512 + 512],
                        xbf[cq][:, 2 * k2:2 * k2 + 2, jl * P:(jl + 1) * P],
                        wbf[q][:, 2 * k2:2 * k2 + 2, :],
                        start=(k2 == 0),
                        stop=(k2 == 3),
                        perf_mode=DR,
                    )

            def x2sq(j):
                """x2c_j = +sum(x_j^2)/D via ACT Square + free-dim accum."""
                scr = spool.tile([P, 1024], bf16, tag="scr", name=f"scr{j}")
                x2c = x2pool.tile([P, 1], f32, tag="x2c", name=f"x2c{j}")
                nc.scalar.activation(scr[:], xrr[j // 4][:, j % 4], Square,
                                     bias=0.0, scale=x2_sqrt_scale,
                                     accum_out=x2c[:])
                x2cs[j] = x2c

            ACT_P1 = {(j, 0) for j in (0, 1, 2, 4, 6, 8, 10, 12, 13, 14, 15)}
            ACT_P1 |= {(0, 1), (1, 1), (2, 1)}

            def p1(j):
                """u = -cross + x2 per half; split ACT/DVE by table."""
                ps0, ps1 = psums.pop(j)
                x2c = x2cs.pop(j)
                th = []
                for h, psh in enumerate((ps0, ps1)):
                    t = epool.tile([P, 1024], f16, tag="t", name=f"t{j}_{h}")
                    if (j, h) in ACT_P1:
                        nc.scalar.activation(t[:], psh[:], Identity,
                                             bias=x2c[:],
                                             scale=-cross_scale)
                    else:
                        nc.vector.tensor_scalar(t[:], psh[:], -cross_scale,
                                                x2c[:], op0=MULT, op1=ADD)
                    th.append(t)
                ttiles[j] = th

            def p2(j, h):
                """y = (-w2) - u on DVE (fp16)."""
                t = ttiles[j][h]
                g = g_of[j]
                if g not in ybufs:
                    ybufs[g] = ypool.tile([P, 4, C], f16, tag="yb",
                                          name=f"yb{g}")
                jo = j - GROUPS[g][0]
                ysl = ybufs[g][:, jo, h * 1024:(h + 1) * 1024]
                w2sl = w2rep[:, h * 1024:(h + 1) * 1024]
                if PASS2 == "scan":
                    nc.vector.tensor_tensor_scan(ysl, w2sl, t[:], 0.0,
                                                 op0=BYP, op1=SUB)
                else:
                    nc.vector.tensor_tensor(ysl, w2sl, t[:], op=SUB)

            def wsq_dve(q):
                nc.vector.tensor_tensor(wsq[q][:], wbf[q][:], wbf[q][:],
                                        op=MULT)

            def wsq_gps(q):
                nc.gpsimd.tensor_tensor(wsq[q][:], wbf[q][:], wbf[q][:],
                                        op=MULT)

            def wsq_act(q):
                nc.scalar.activation(wsq[q][:], wbf[q][:], Square)

            w2ps = {}

            def w2red(q):
                wp = pmain.tile([P, 1024], f32, tag="ps", name=f"w2ps{q}")
                for k2 in range(4):
                    nc.tensor.matmul(
                        wp[0:16, 0:512],
                        negones_dr[:],
                        wsq[q][:, 2 * k2:2 * k2 + 2, :],
                        start=(k2 == 0),
                        stop=(k2 == 3),
                        perf_mode=DR,
                    )
                w2ps[q] = wp

            def w2row_q(q):
                wp = w2ps.pop(q)
                nc.scalar.activation(w2row[:, q * 512:(q + 1) * 512],
                                     wp[0:1, 0:512], Copy, bias=0.0,
                                     scale=w2_scale)

            w2rp = {}

            def w2rp_q(q):
                wp = pmain.tile([P, 1024], f32, tag="ps", name=f"w2rp{q}")
                nc.tensor.matmul(wp[:, 0:512], ones1_b[:],
                                 w2row[:, q * 512:(q + 1) * 512],
                                 start=True, stop=True)
                w2rp[q] = wp

            def w2rep_q(q):
                wp = w2rp.pop(q)
                nc.scalar.activation(w2rep[:, q * 512:(q + 1) * 512],
                                     wp[:, 0:512], Copy, bias=0.0, scale=1.0)

            def store(g):
                j0, nj = GROUPS[g]
                yb = ybufs.pop(g)
                nc.sync.dma_start(yD[:, j0:j0 + nj, :], yb[:, 0:nj, :])

            def store_h(g, h):
                j0, nj = GROUPS[g]
                yb = ybufs[g] if h == 0 else ybufs.pop(g)
                nc.sync.dma_start(
                    yD[:, j0:j0 + nj, h * 1024:(h + 1) * 1024],
                    yb[:, 0:nj, h * 1024:(h + 1) * 1024],
                )

            # ---- scheduled emission ----
            wsq_act(0)
            wsq_gps(2); wsq_gps(3)
            mms(0, q_outer=True)
            x2sq(0)
            p1(0)
            mms(1)
            x2sq(1); x2sq(2); x2sq(3)
            p1(1)
            mms(2)
            p1(2)
            wsq_act(1)
            mms(3)
            x2sq(4)
            w2red(0); w2row_q(0)
            p1(3)
            mms(4)
            x2sq(5)
            w2red(1); w2row_q(1)
            w2rp_q(0); w2rep_q(0)
            p1(4)
            mms(5)
            x2sq(6)
            w2red(2); w2row_q(2)
            w2rp_q(1); w2rep_q(1)
            p1(5)
            p2(0, 0); p2(1, 0)
            mms(6)
            x2sq(7)
            w2red(3); w2row_q(3)
            w2rp_q(2); w2rep_q(2)
            p1(6)
            p2(2, 0); p2(3, 0); p2(4, 0)
            mms(7)
            x2sq(8)
            w2rp_q(3); w2rep_q(3)
            p1(7)
            p2(5, 0); p2(6, 0); p2(0, 1); p2(1, 1)
            mms(8)
            x2sq(9)
            p1(8)
            p2(7, 0); p2(2, 1); p2(3, 1)
            store(0)
            mms(9)
            x2sq(10)
            p1(9)
            p2(8, 0); p2(8, 1); p2(4, 1); p2(5, 1)
            mms(10)
            x2sq(11)
            p1(10)
            p2(9, 0); p2(9, 1); p2(6, 1); p2(7, 1)
            store(1)
            mms(11)
            x2sq(12)
            p1(11)
            p2(10, 0); p2(10, 1)
            store(2)
            mms(12)
            x2sq(13)
            p1(12)
            p2(11, 0); p2(11, 1)
            store(3)
            mms(13)
            x2sq(14)
            p1(13)
            p2(12, 0); p2(12, 1)
            mms(14)
            x2sq(15)
            p1(14)
            p2(13, 0); p2(13, 1)
            store(4)
            mms(15)
            p2(14, 0)
            store_h(5, 0)
            p2(14, 1)
            store_h(5, 1)
            # tile 15 epilogue at quarter granularity: ACT p1 quarters feed
            # DVE p2 quarters concurrently to shorten the drain tail
            ps0_15, ps1_15 = psums.pop(15)
            x2c_15 = x2cs.pop(15)
            yb5 = ypool.tile([P, 4, C], f16, tag="yb", name="yb5")
            for qq in range(4):
                psh = (ps0_15, ps1_15)[qq // 2]
                sl = slice((qq % 2) * 512, (qq % 2) * 512 + 512)
                csl = slice(qq * 512, (qq + 1) * 512)
                tq = epool.tile([P, 512], f16, tag="tq", bufs=4,
                                name=f"t15_{qq}")
                nc.scalar.activation(tq[:], psh[:, sl], Identity,
                                     bias=x2c_15[:], scale=-cross_scale)
                nc.vector.tensor_tensor(yb5[:, 0, csl], w2rep[:, csl],
                                        tq[:], op=SUB)
            nc.sync.dma_start(yD[:, 15:16, 0:1024], yb5[:, 0:1, 0:1024])
            nc.sync.dma_start(yD[:, 15:16, 1024:2048], yb5[:, 0:1, 1024:2048])

    nc.compile()
    return nc


def _get_nc():
    if "nc" not in _CACHE:
        _CACHE["nc"] = _build_nc()
    return _CACHE["nc"]


def _prep_inputs(x, W):
    x = np.ascontiguousarray(x, dtype=np.float32)
    W = np.ascontiguousarray(W, dtype=np.float32)
    e4m3 = ml_dtypes.float8_e4m3
    bf16 = ml_dtypes.bfloat16
    # wP[p, q, k, c'] = 16*W[q*512+c', k*128+p]
    w8 = (W * np.float32(16.0)).astype(e4m3)
    wPm = np.ascontiguousarray(
        w8.reshape(4, 512, 8, P).transpose(3, 0, 2, 1)
    )
    in_maps = []
    for i in range(NCORES):
        xs = x[i * BSH:(i + 1) * BSH]
        x8 = xs.astype(e4m3)
        # xP[p, c, k, b'] = x8[c*512+b', k*128+p]
        xPm = np.ascontiguousarray(
            x8.T.reshape(8, P, 4, 512).transpose(1, 2, 0, 3)
        )
        xb = xs.astype(bf16)
        # xR[p, j, d] = xb[j*128+p, d]
        xRm = np.ascontiguousarray(
            xb.reshape(NT, P, D).transpose(1, 0, 2)
        )
        in_maps.append({"xP": xPm, "xR": xRm, "wP": wPm})
    return in_maps


def run(x, W, trace=False, **trace_kwargs):
    """Run on the 8 cores; returns (out [B, C] f32, BassKernelResults)."""
    from concourse import bass_utils

    nc = _get_nc()
    in_maps = _prep_inputs(x, W)
    res = bass_utils.run_bass_kernel_spmd(
        nc, in_maps, core_ids=list(range(NCORES)), trace=trace, **trace_kwargs
    )
    outs = []
    for r in res.results:
        yt = r["y"]  # [128, 16, 2048] fp16
        outs.append(
            np.ascontiguousarray(yt.transpose(1, 0, 2))
            .reshape(BSH, C)
            .astype(np.float32)
        )
    out = np.concatenate(outs, axis=0)
    return out, res


def kernel(x, W, task_id=None, **_unused):
    out, _ = run(np.asarray(x), np.asarray(W), trace=False)
    return out
